# revision 3
# baseline (speedup 1.0000x reference)
"""Trainium2 Bass kernel v2 for nn_AttentiveTransformer (Dense + BN + prior
mask + sparsemax).

Wire formats: x [D_IN, B] f16; priors u8 (round(p*255), partition-major);
output u8 (= round(255*out), saturating round-to-nearest-even on the
engines); W'/bias f16. All sparsemax math runs in the 255x domain:
z' = (xW'+b) * p_u8 = 255*z exactly, tau' = (S'_k - 255)/k.

Per pair of tiles (one PSUM bank, [128, 512] f32):
    PE:   512-wide bias matmul (ones^T @ [bp|bp]) + two x^T@W' accumulate
    pass1 (z' = psum * priors): DVE tensor_tensor directly from PSUM, or
          ACT copy PSUM->SBUF + GPSIMD tensor_tensor (engine per pair idx,
          env K2_MULT pattern)
    DVE:  per 128-half, max8 sorted top-8 straight into stats slots
          [s0+1..s0+8], [s0+10..s0+17] (SLOTS=18, slots 0/9 zeroed once)
Per group of GSIZE=16 tiles (batched threshold math, split-pair formula):
    scan: segmented cumsum -> A_i, B_j prefix sums (slot 0 resets)
    TT:   pairs = A_i + B_j broadcast [9x9]
    STT:  ntaus = (pairs - 255) * (-1/(i+j))
    reduce: ntau = min_k ntaus  (= -tau')
    each of scan/TT/STT/reduce on DVE or GPSIMD via env K2_SCAN/W3/W4/W5
    pass2: out_u8 = sat_round(z' + ntau), engine per tile idx (K2_PASS2):
          ACT activation(Relu,bias) / DVE tensor_scalar / GPSIMD tensor_scalar
Host: x transposed+f16, priors quantized u8 partition-major; output u8
inverse-permuted, /255 -> f32. Max rel err vs fp32 reference ~1.2e-2
(u8 priors quantization + top-8-per-half truncation), gate 2e-2.
"""
import os
import sys

sys.path.insert(0, "/opt/trn_rl_repo")

import numpy as np

import concourse.bass as bass
import concourse.mybir as mybir
from concourse.tile import TileContext

F32 = mybir.dt.float32
F16 = mybir.dt.float16
U8 = mybir.dt.uint8
ALU = mybir.AluOpType
ACTF = mybir.ActivationFunctionType
F16NP = np.float16

N_CORES = 8
B = 262144
D_IN = 128
D_OUT = 256
BC = B // N_CORES
GSIZE = 16
SLOTS = 18

# engine assignment knobs
MULT_PAT = os.environ.get("K2_MULT", "dggdgggg")       # per pair: d|g (g = ACT copy + GPSIMD mult)
PASS2_PAT = os.environ.get("K2_PASS2", "aaaaaaaaaaaaaaaa")  # per tile: a|d|g
SCAN_ENG = os.environ.get("K2_SCAN", "d")
W3_ENG = os.environ.get("K2_W3", "g")                  # pairs TT (add)
W4_ENG = os.environ.get("K2_W4", "d")                  # ntaus STT
W5_ENG = os.environ.get("K2_W5", "d")                  # min reduce
ZBUFS = int(os.environ.get("K2_ZBUFS", str(GSIZE + 3)))
PSUMB = int(os.environ.get("K2_PSUMB", "8"))
PIPE = int(os.environ.get("K2_PIPE", "0"))


def _split_oversized_waits(nc, max_waits=1):
    """walrus setupSyncWait rejects instructions with many sem waits; split
    the excess onto same-engine Drain instructions placed just before."""
    for f in nc.m.functions:
        for bb in f.blocks:
            insts = bb.instructions
            i = 0
            while i < len(insts):
                inst = insts[i]
                si = inst.sync_info
                waits = list(si.on_wait) if si and si.on_wait else []
                if len(waits) > max_waits:
                    si.on_wait = waits[:max_waits]
                    rest = waits[max_waits:]
                    pos = i
                    for j in range(0, len(rest), max_waits):
                        d = mybir.InstDrain(
                            name=f"{inst.name}_wsplit{j}", ins=[], outs=[],
                            bass_is_fusable=False,
                        )
                        d.engine = inst.engine
                        d.sync_info = mybir.SyncInfo(
                            on_wait=rest[j:j + max_waits], on_update=[])
                        insts.insert(pos, d)
                        pos += 1
                        i += 1
                i += 1


def _veng(nc, c):
    return nc.vector if c == "d" else nc.gpsimd


def build_nc(bc=BC):
    assert bc % 128 == 0
    n_tiles = bc // 128
    assert n_tiles % GSIZE == 0

    nc = bass.Bass()
    xin = nc.declare_dram_parameter("xin", [D_IN, bc], F16, isOutput=False)
    prin = nc.declare_dram_parameter("prin", [128, (bc // 128) * D_OUT], U8,
                                     isOutput=False)
    wp = nc.declare_dram_parameter("wp", [D_IN, D_OUT], F16, isOutput=False)
    bp = nc.declare_dram_parameter("bp", [1, D_OUT], F16, isOutput=False)
    ones = nc.declare_dram_parameter("ones", [1, D_IN], F16, isOutput=False)
    jc = nc.declare_dram_parameter("jc", [128, GSIZE * 81], F32, isOutput=False)
    sm = nc.declare_dram_parameter("sm", [128, GSIZE * SLOTS], F32,
                                   isOutput=False)
    out = nc.declare_dram_parameter("out", [128, (bc // 128) * D_OUT], U8,
                                    isOutput=True)

    xin_c = xin[:, :]
    prin_t = prin[:, :]
    out_t = out[:, :]

    # group schedule: small ramp-up/down groups shrink pipeline fill/drain
    mid = (n_tiles - 16) // GSIZE
    assert mid * GSIZE + 16 == n_tiles
    sizes = [8] + [GSIZE] * mid + [4, 4]
    schedule = []
    t = 0
    for s in sizes:
        schedule.append((t, s))
        t += s

    with TileContext(nc) as tc:
        with (
            tc.tile_pool(name="const", bufs=1) as constp,
            tc.tile_pool(name="xload", bufs=3) as xloadp,
            tc.tile_pool(name="pload", bufs=3) as ploadp,
            tc.tile_pool(name="z", bufs=ZBUFS) as zp,
            tc.tile_pool(name="zc", bufs=8) as zcp,
            tc.tile_pool(name="outs", bufs=3) as outsp,
            tc.tile_pool(name="stats", bufs=3) as statsp,
            tc.tile_pool(name="small", bufs=3) as smallp,
            tc.tile_pool(name="psz", bufs=PSUMB, space="PSUM") as psumz,
        ):
            wp_sb = constp.tile([D_IN, D_OUT], F16)
            nc.sync.dma_start(out=wp_sb[:], in_=wp[:, :])
            bp2_sb = constp.tile([1, 2 * D_OUT], F16)
            nc.sync.dma_start(out=bp2_sb[:, 0:D_OUT], in_=bp[:, :])
            nc.sync.dma_start(out=bp2_sb[:, D_OUT:2 * D_OUT], in_=bp[:, :])
            ones_sb = constp.tile([1, D_IN], F16)
            nc.sync.dma_start(out=ones_sb[:], in_=ones[:, :])
            jc_sb = constp.tile([128, GSIZE * 81], F32)
            sm_sb = constp.tile([128, GSIZE * SLOTS], F32)

            def emit_group_math(prev):
                (gt0, gs), ztiles_p, og_p, ntau_p, stats_p, cums_p, \
                    pairs_p, ntaus_p = prev
                _veng(nc, SCAN_ENG).tensor_tensor_scan(
                    cums_p[:, 0:gs * SLOTS], sm_sb[:, 0:gs * SLOTS],
                    stats_p[:, 0:gs * SLOTS], 0.0, ALU.mult, ALU.add)
                cv = cums_p[:, 0:gs * SLOTS].rearrange(
                    "p (t s) -> p t s", s=SLOTS)
                a4 = cv[:, :, 0:9].rearrange("p t (i u) -> p t i u", u=1)
                b4 = cv[:, :, 9:18].rearrange("p t (u j) -> p t u j", u=1)
                a4b, b4b = bass.broadcast_tensor_aps(a4, b4)
                pv = pairs_p[:, 0:gs * 81].rearrange(
                    "p (t i j) -> p t i j", i=9, j=9)
                _veng(nc, W3_ENG).tensor_tensor(pv, a4b, b4b, ALU.add)
                _veng(nc, W4_ENG).scalar_tensor_tensor(
                    ntaus_p[:, 0:gs * 81], pairs_p[:, 0:gs * 81], 255.0,
                    jc_sb[:, 0:gs * 81], ALU.subtract, ALU.mult)
                nv = ntaus_p[:, 0:gs * 81].rearrange(
                    "p (t k) -> p t k", k=81)[:, :, 1:81]
                _veng(nc, W5_ENG).tensor_reduce(
                    ntau_p[:, 0:gs], nv, mybir.AxisListType.X, ALU.min)

            def math_actions(prev):
                (gt0, gs), ztiles_p, og_p, ntau_p, stats_p, cums_p, \
                    pairs_p, ntaus_p = prev

                def a_scan():
                    _veng(nc, SCAN_ENG).tensor_tensor_scan(
                        cums_p[:, 0:gs * SLOTS], sm_sb[:, 0:gs * SLOTS],
                        stats_p[:, 0:gs * SLOTS], 0.0, ALU.mult, ALU.add)

                def a_tt():
                    cv = cums_p[:, 0:gs * SLOTS].rearrange(
                        "p (t s) -> p t s", s=SLOTS)
                    a4 = cv[:, :, 0:9].rearrange("p t (i u) -> p t i u", u=1)
                    b4 = cv[:, :, 9:18].rearrange("p t (u j) -> p t u j", u=1)
                    a4b, b4b = bass.broadcast_tensor_aps(a4, b4)
                    pv = pairs_p[:, 0:gs * 81].rearrange(
                        "p (t i j) -> p t i j", i=9, j=9)
                    _veng(nc, W3_ENG).tensor_tensor(pv, a4b, b4b, ALU.add)

                def a_stt():
                    _veng(nc, W4_ENG).scalar_tensor_tensor(
                        ntaus_p[:, 0:gs * 81], pairs_p[:, 0:gs * 81], 255.0,
                        jc_sb[:, 0:gs * 81], ALU.subtract, ALU.mult)

                def a_red():
                    nv = ntaus_p[:, 0:gs * 81].rearrange(
                        "p (t k) -> p t k", k=81)[:, :, 1:81]
                    _veng(nc, W5_ENG).tensor_reduce(
                        ntau_p[:, 0:gs], nv, mybir.AxisListType.X, ALU.min)

                return [a_scan, a_tt, a_stt, a_red]

            def pass2_actions(prev):
                (gt0, gs), ztiles_p, og_p, ntau_p = prev[:4]
                h1 = (gs // 2) & ~1
                acts = []
                for t0, z_sb in ztiles_p:
                    for h in range(2):
                        t = t0 + h

                        def a_relu(t=t, z_sb=z_sb, h=h):
                            eng = PASS2_PAT[t % len(PASS2_PAT)]
                            zt = z_sb[:, h * D_OUT:(h + 1) * D_OUT]
                            if eng == "a":
                                nc.scalar.activation(
                                    og_p[:, t, :], zt, ACTF.Relu,
                                    bias=ntau_p[:, t:t + 1], scale=1.0)
                            else:
                                _veng(nc, eng).tensor_scalar(
                                    og_p[:, t, :], zt, ntau_p[:, t:t + 1],
                                    0.0, ALU.add, ALU.bypass)
                        acts.append(a_relu)
                    if h1 and t0 + 2 == h1:
                        def a_store1():
                            nc.sync.dma_start(
                                out=out_t[:, gt0 * D_OUT:(gt0 + h1) * D_OUT],
                                in_=og_p[:, 0:h1, :].rearrange(
                                    "p t d -> p (t d)"))
                        acts.append(a_store1)

                def a_store2():
                    nc.sync.dma_start(
                        out=out_t[:, (gt0 + h1) * D_OUT:(gt0 + gs) * D_OUT],
                        in_=og_p[:, h1:gs, :].rearrange("p t d -> p (t d)"))
                acts.append(a_store2)
                return acts

            prev_group = None
            for gi in range(len(schedule)):
                gt0, gs = schedule[gi]
                n_pairs = gs // 2
                xg = xloadp.tile([128, GSIZE * 128], F16, tag="xg")
                nc.sync.dma_start(out=xg[:, 0:gs * 128],
                                  in_=xin_c[:, gt0 * 128:(gt0 + gs) * 128])
                pg = ploadp.tile([128, GSIZE, D_OUT], U8, tag="pg")
                nc.sync.dma_start(
                    out=pg[:, 0:gs, :].rearrange("p t d -> p (t d)"),
                    in_=prin_t[:, gt0 * D_OUT:(gt0 + gs) * D_OUT])
                if gi == 0:
                    nc.sync.dma_start(out=jc_sb[:], in_=jc[:, :])
                    nc.sync.dma_start(out=sm_sb[:], in_=sm[:, :])
                og = outsp.tile([128, GSIZE, D_OUT], U8)

                stats = statsp.tile([128, GSIZE * SLOTS], F32)
                cums = statsp.tile([128, GSIZE * SLOTS], F32, tag="cums")
                pairs = statsp.tile([128, GSIZE * 81], F32, tag="pairs")
                ntaus = statsp.tile([128, GSIZE * 81], F32, tag="ntaus")
                ntau = smallp.tile([128, GSIZE], F32, tag="ntau")
                if gi < 3:
                    # zero slots 0/9 of every tile segment once per ring
                    # buffer (stats pool has 3 bufs); never written again
                    nc.gpsimd.memset(stats[:], 0.0)

                # interleave schedule for prev-group actions: after pair 0
                # emit scan+TT, after pair 1 STT+reduce, then pass2 chunks
                prev_math = math_actions(prev_group) if (
                    PIPE and prev_group is not None) else []
                prev_p2 = pass2_actions(prev_group) if (
                    PIPE and prev_group is not None) else []

                stt_at = min(3, n_pairs - 1)

                def run_chunk(pr):
                    if not PIPE or (not prev_math and not prev_p2):
                        return
                    if pr == 0:
                        # scan (DVE) + pairs-TT (GPSIMD) early so the TT
                        # result is ready when the STT needs it
                        for a in prev_math[0:2]:
                            a()
                        del prev_math[0:2]
                    elif pr == stt_at:
                        for a in prev_math:
                            a()
                        prev_math.clear()
                    elif pr > stt_at:
                        k = max(1, (len(prev_p2) + n_pairs - pr - 1)
                                // (n_pairs - pr))
                        for a in prev_p2[0:k]:
                            a()
                        del prev_p2[0:k]

                ztiles = []
                for pr in range(n_pairs):
                    t0 = 2 * pr
                    z_ps = psumz.tile([128, 2 * D_OUT], F32)
                    nc.tensor.matmul(z_ps[:], ones_sb[:], bp2_sb[:],
                                     start=True, stop=False)
                    nc.tensor.matmul(z_ps[:, 0:D_OUT],
                                     xg[:, t0 * 128:(t0 + 1) * 128],
                                     wp_sb[:], start=False, stop=True,
                                     skip_group_check=True)
                    nc.tensor.matmul(z_ps[:, D_OUT:2 * D_OUT],
                                     xg[:, (t0 + 1) * 128:(t0 + 2) * 128],
                                     wp_sb[:], start=False, stop=True,
                                     skip_group_check=True)

                    z_sb = zp.tile([128, 2 * D_OUT], F32)
                    pgp = pg[:, t0:t0 + 2, :].rearrange("p t d -> p (t d)")
                    mode = MULT_PAT[pr % len(MULT_PAT)]
                    if mode == "d":
                        nc.vector.tensor_tensor(z_sb[:], z_ps[:], pgp,
                                                ALU.mult)
                    else:
                        zc = zcp.tile([128, 2 * D_OUT], F32, tag="zc")
                        if mode == "m":
                            nc.sync.dma_start(out=zc[:], in_=z_ps[:])
                        else:
                            nc.scalar.copy(zc[:], z_ps[:])
                        nc.gpsimd.tensor_tensor(z_sb[:], zc[:], pgp, ALU.mult)

                    for h in range(2):
                        t = t0 + h
                        zt = z_sb[:, h * D_OUT:(h + 1) * D_OUT]
                        s0 = t * SLOTS
                        nc.vector.max(stats[:, s0 + 1:s0 + 9], zt[:, 0:128])
                        nc.vector.max(stats[:, s0 + 10:s0 + 18],
                                      zt[:, 128:256])
                    ztiles.append((t0, z_sb))
                    run_chunk(pr)

                for a in prev_math:
                    a()
                for a in prev_p2:
                    a()

                cur_group = ((gt0, gs), ztiles, og, ntau,
                             stats, cums, pairs, ntaus)
                if PIPE:
                    prev_group = cur_group
                else:
                    for a in math_actions(cur_group):
                        a()
                    for a in pass2_actions(cur_group):
                        a()

            if PIPE and prev_group is not None:
                for a in math_actions(prev_group):
                    a()
                for a in pass2_actions(prev_group):
                    a()

    _split_oversized_waits(nc)
    return nc


def _host_constants(W, gamma, beta, moving_mean, moving_var):
    inv = (gamma / np.sqrt(moving_var + 1e-3)).astype(np.float32)
    wp = (W * inv[None, :]).astype(F16NP)
    bp = (beta - moving_mean * inv).astype(F16NP).reshape(1, D_OUT)
    ones = np.ones((1, D_IN), dtype=F16NP)
    # jc[i, j] = -1/(i+j); (0,0) slot excluded by the reduce
    ij = np.add.outer(np.arange(9), np.arange(9)).astype(np.float32)
    ij[0, 0] = 1.0
    jrow = (-1.0 / ij).reshape(81).astype(np.float32)
    jrow[0] = 0.0
    jrow = np.tile(jrow, GSIZE)
    srow = np.tile(
        np.concatenate([[0.0], np.ones(8), [0.0], np.ones(8)]),
        GSIZE).astype(np.float32)
    jct = np.ascontiguousarray(np.broadcast_to(jrow, (128, len(jrow))),
                               dtype=np.float32)
    smt = np.ascontiguousarray(np.broadcast_to(srow, (128, len(srow))),
                               dtype=np.float32)
    return wp, bp, ones, jct, smt


_NC_CACHE = {}


def make_core_feeds(inputs, priors, W, gamma, beta, moving_mean, moving_var,
                    bc=BC, n_cores=N_CORES):
    inputs_t = np.ascontiguousarray(
        np.asarray(inputs, dtype=np.float32).T).astype(F16NP)  # [D_IN, B]
    pq = np.round(np.asarray(priors, dtype=np.float32) * 255.0).astype(np.uint8)
    n_tiles = bc // 128
    wp, bp, ones, jct, smt = _host_constants(
        np.asarray(W, dtype=np.float32), np.asarray(gamma, dtype=np.float32),
        np.asarray(beta, dtype=np.float32),
        np.asarray(moving_mean, dtype=np.float32),
        np.asarray(moving_var, dtype=np.float32))
    in_maps = []
    for c in range(n_cores):
        lo, hi = c * bc, (c + 1) * bc
        pr = np.ascontiguousarray(
            pq[lo:hi].reshape(n_tiles, 128, D_OUT).transpose(1, 0, 2)
        ).reshape(128, n_tiles * D_OUT)
        in_maps.append({
            "xin": np.ascontiguousarray(inputs_t[:, lo:hi]),
            "prin": pr,
            "wp": wp, "bp": bp, "ones": ones, "jc": jct, "sm": smt,
        })
    return in_maps


def kernel(inputs, priors, W, gamma, beta, moving_mean, moving_var):
    from concourse.bass_utils import run_bass_kernel_spmd

    in_maps = make_core_feeds(inputs, priors, W, gamma, beta,
                              moving_mean, moving_var)
    if BC not in _NC_CACHE:
        _NC_CACHE[BC] = build_nc(BC)
    nc = _NC_CACHE[BC]
    res = run_bass_kernel_spmd(nc, in_maps, list(range(N_CORES)))
    n_tiles = BC // 128
    parts = []
    inv255 = np.float32(1.0 / 255.0)
    for c in range(N_CORES):
        o = res.results[c]["out"].reshape(128, n_tiles, D_OUT)
        parts.append(
            o.transpose(1, 0, 2).reshape(BC, D_OUT).astype(np.float32) * inv255)
    return np.concatenate(parts, axis=0)


# revision 5
# speedup vs baseline: 1.0462x; 1.0462x over previous
"""Trainium2 Bass kernel v2 for nn_AttentiveTransformer (Dense + BN + prior
mask + sparsemax).

Wire formats: x [D_IN, B] f16; priors u8 (round(p*255), partition-major);
output u8 (= round(255*out), saturating round-to-nearest-even on the
engines); W'/bias f16. All sparsemax math runs in the 255x domain:
z' = (xW'+b) * p_u8 = 255*z exactly, tau' = (S'_k - 255)/k.

Per pair of tiles (one PSUM bank, [128, 512] f32):
    PE:   512-wide bias matmul (ones^T @ [bp|bp]) + two x^T@W' accumulate
    pass1 (z' = psum * priors): DVE tensor_tensor directly from PSUM, or
          ACT copy PSUM->SBUF + GPSIMD tensor_tensor (engine per pair idx,
          env K2_MULT pattern)
    DVE:  per 128-half, max8 sorted top-8 straight into stats slots
          [s0+1..s0+8], [s0+10..s0+17] (SLOTS=18, slots 0/9 zeroed once)
Per group of GSIZE=16 tiles (batched threshold math, split-pair formula):
    scan: segmented cumsum -> A_i, B_j prefix sums (slot 0 resets)
    TT:   pairs = A_i + B_j broadcast [9x9]
    STT:  ntaus = (pairs - 255) * (-1/(i+j))
    reduce: ntau = min_k ntaus  (= -tau')
    each of scan/TT/STT/reduce on DVE or GPSIMD via env K2_SCAN/W3/W4/W5
    pass2: out_u8 = sat_round(z' + ntau), engine per tile idx (K2_PASS2):
          ACT activation(Relu,bias) / DVE tensor_scalar / GPSIMD tensor_scalar
Host: x transposed+f16, priors quantized u8 partition-major; output u8
inverse-permuted, /255 -> f32. Max rel err vs fp32 reference ~1.2e-2
(u8 priors quantization + top-8-per-half truncation), gate 2e-2.
"""
import os
import sys

sys.path.insert(0, "/opt/trn_rl_repo")

import numpy as np

import concourse.bass as bass
import concourse.mybir as mybir
from concourse.tile import TileContext

F32 = mybir.dt.float32
F16 = mybir.dt.float16
U8 = mybir.dt.uint8
ALU = mybir.AluOpType
ACTF = mybir.ActivationFunctionType
F16NP = np.float16

N_CORES = 8
B = 262144
D_IN = 128
D_OUT = 256
BC = B // N_CORES
GSIZE = 16
SLOTS = 18

# engine assignment knobs
MULT_PAT = os.environ.get("K2_MULT", "dggdgggg")       # per pair: d|g (g = ACT copy + GPSIMD mult)
PASS2_PAT = os.environ.get("K2_PASS2", "aaaaaaaaaaaaaaaa")  # per tile: a|d|g
SCAN_ENG = os.environ.get("K2_SCAN", "d")
W3_ENG = os.environ.get("K2_W3", "g")                  # pairs TT (add)
W4_ENG = os.environ.get("K2_W4", "d")                  # ntaus STT
W5_ENG = os.environ.get("K2_W5", "d")                  # min reduce
ZBUFS = int(os.environ.get("K2_ZBUFS", str(GSIZE + 3)))
PSUMB = int(os.environ.get("K2_PSUMB", "8"))
PIPE = int(os.environ.get("K2_PIPE", "0"))


def _split_oversized_waits(nc, max_waits=1):
    """walrus setupSyncWait rejects instructions with many sem waits; split
    the excess onto same-engine Drain instructions placed just before."""
    for f in nc.m.functions:
        for bb in f.blocks:
            insts = bb.instructions
            i = 0
            while i < len(insts):
                inst = insts[i]
                si = inst.sync_info
                waits = list(si.on_wait) if si and si.on_wait else []
                if len(waits) > max_waits:
                    si.on_wait = waits[:max_waits]
                    rest = waits[max_waits:]
                    pos = i
                    for j in range(0, len(rest), max_waits):
                        d = mybir.InstDrain(
                            name=f"{inst.name}_wsplit{j}", ins=[], outs=[],
                            bass_is_fusable=False,
                        )
                        d.engine = inst.engine
                        d.sync_info = mybir.SyncInfo(
                            on_wait=rest[j:j + max_waits], on_update=[])
                        insts.insert(pos, d)
                        pos += 1
                        i += 1
                i += 1


def _veng(nc, c):
    return nc.vector if c == "d" else nc.gpsimd


def build_nc(bc=BC):
    assert bc % 128 == 0
    n_tiles = bc // 128
    assert n_tiles % GSIZE == 0

    nc = bass.Bass()
    xin = nc.declare_dram_parameter("xin", [D_IN, bc], F16, isOutput=False)
    prin = nc.declare_dram_parameter("prin", [128, (bc // 128) * D_OUT], U8,
                                     isOutput=False)
    wp = nc.declare_dram_parameter("wp", [D_IN, D_OUT], F16, isOutput=False)
    bp = nc.declare_dram_parameter("bp", [1, D_OUT], F16, isOutput=False)
    ones = nc.declare_dram_parameter("ones", [1, D_IN], F16, isOutput=False)
    jc = nc.declare_dram_parameter("jc", [128, GSIZE * 81], F32, isOutput=False)
    sm = nc.declare_dram_parameter("sm", [128, GSIZE * SLOTS], F32,
                                   isOutput=False)
    out = nc.declare_dram_parameter("out", [128, (bc // 128) * D_OUT], U8,
                                    isOutput=True)

    xin_c = xin[:, :]
    prin_t = prin[:, :]
    out_t = out[:, :]

    # group schedule: small ramp-up/down groups shrink pipeline fill/drain
    mid = (n_tiles - 16) // GSIZE
    assert mid * GSIZE + 16 == n_tiles
    sizes = [8] + [GSIZE] * mid + [4, 4]
    schedule = []
    t = 0
    for s in sizes:
        schedule.append((t, s))
        t += s

    with TileContext(nc) as tc:
        with (
            tc.tile_pool(name="const", bufs=1) as constp,
            tc.tile_pool(name="xload", bufs=3) as xloadp,
            tc.tile_pool(name="pload", bufs=3) as ploadp,
            tc.tile_pool(name="z", bufs=ZBUFS) as zp,
            tc.tile_pool(name="zc", bufs=8) as zcp,
            tc.tile_pool(name="outs", bufs=3) as outsp,
            tc.tile_pool(name="stats", bufs=3) as statsp,
            tc.tile_pool(name="small", bufs=3) as smallp,
            tc.tile_pool(name="psz", bufs=PSUMB, space="PSUM") as psumz,
        ):
            wp_sb = constp.tile([D_IN, D_OUT], F16)
            nc.sync.dma_start(out=wp_sb[:], in_=wp[:, :])
            bp2_sb = constp.tile([1, 2 * D_OUT], F16)
            nc.sync.dma_start(out=bp2_sb[:, 0:D_OUT], in_=bp[:, :])
            nc.sync.dma_start(out=bp2_sb[:, D_OUT:2 * D_OUT], in_=bp[:, :])
            ones_sb = constp.tile([1, D_IN], F16)
            nc.sync.dma_start(out=ones_sb[:], in_=ones[:, :])
            jc_sb = constp.tile([128, GSIZE * 81], F32)
            sm_sb = constp.tile([128, GSIZE * SLOTS], F32)

            def emit_group_math(prev):
                (gt0, gs), ztiles_p, og_p, ntau_p, stats_p, cums_p, \
                    pairs_p, ntaus_p = prev
                _veng(nc, SCAN_ENG).tensor_tensor_scan(
                    cums_p[:, 0:gs * SLOTS], sm_sb[:, 0:gs * SLOTS],
                    stats_p[:, 0:gs * SLOTS], 0.0, ALU.mult, ALU.add)
                cv = cums_p[:, 0:gs * SLOTS].rearrange(
                    "p (t s) -> p t s", s=SLOTS)
                a4 = cv[:, :, 0:9].rearrange("p t (i u) -> p t i u", u=1)
                b4 = cv[:, :, 9:18].rearrange("p t (u j) -> p t u j", u=1)
                a4b, b4b = bass.broadcast_tensor_aps(a4, b4)
                pv = pairs_p[:, 0:gs * 81].rearrange(
                    "p (t i j) -> p t i j", i=9, j=9)
                _veng(nc, W3_ENG).tensor_tensor(pv, a4b, b4b, ALU.add)
                _veng(nc, W4_ENG).scalar_tensor_tensor(
                    ntaus_p[:, 0:gs * 81], pairs_p[:, 0:gs * 81], 255.0,
                    jc_sb[:, 0:gs * 81], ALU.subtract, ALU.mult)
                nv = ntaus_p[:, 0:gs * 81].rearrange(
                    "p (t k) -> p t k", k=81)[:, :, 1:81]
                _veng(nc, W5_ENG).tensor_reduce(
                    ntau_p[:, 0:gs], nv, mybir.AxisListType.X, ALU.min)

            def math_actions(prev):
                (gt0, gs), ztiles_p, og_p, ntau_p, stats_p, cums_p, \
                    pairs_p, ntaus_p = prev

                def a_scan():
                    _veng(nc, SCAN_ENG).tensor_tensor_scan(
                        cums_p[:, 0:gs * SLOTS], sm_sb[:, 0:gs * SLOTS],
                        stats_p[:, 0:gs * SLOTS], 0.0, ALU.mult, ALU.add)

                def a_tt():
                    cv = cums_p[:, 0:gs * SLOTS].rearrange(
                        "p (t s) -> p t s", s=SLOTS)
                    a4 = cv[:, :, 0:9].rearrange("p t (i u) -> p t i u", u=1)
                    b4 = cv[:, :, 9:18].rearrange("p t (u j) -> p t u j", u=1)
                    a4b, b4b = bass.broadcast_tensor_aps(a4, b4)
                    pv = pairs_p[:, 0:gs * 81].rearrange(
                        "p (t i j) -> p t i j", i=9, j=9)
                    _veng(nc, W3_ENG).tensor_tensor(pv, a4b, b4b, ALU.add)

                def a_stt():
                    _veng(nc, W4_ENG).scalar_tensor_tensor(
                        ntaus_p[:, 0:gs * 81], pairs_p[:, 0:gs * 81], 255.0,
                        jc_sb[:, 0:gs * 81], ALU.subtract, ALU.mult)

                def a_red():
                    nv = ntaus_p[:, 0:gs * 81].rearrange(
                        "p (t k) -> p t k", k=81)[:, :, 1:81]
                    _veng(nc, W5_ENG).tensor_reduce(
                        ntau_p[:, 0:gs], nv, mybir.AxisListType.X, ALU.min)

                return [a_scan, a_tt, a_stt, a_red]

            def pass2_actions(prev):
                (gt0, gs), ztiles_p, og_p, ntau_p = prev[:4]
                h1 = (gs // 2) & ~1
                acts = []
                for t0, z_sb in ztiles_p:
                    for h in range(2):
                        t = t0 + h

                        def a_relu(t=t, z_sb=z_sb, h=h):
                            eng = PASS2_PAT[t % len(PASS2_PAT)]
                            zt = z_sb[:, h * D_OUT:(h + 1) * D_OUT]
                            if eng == "a":
                                nc.scalar.activation(
                                    og_p[:, t, :], zt, ACTF.Relu,
                                    bias=ntau_p[:, t:t + 1], scale=1.0)
                            else:
                                _veng(nc, eng).tensor_scalar(
                                    og_p[:, t, :], zt, ntau_p[:, t:t + 1],
                                    0.0, ALU.add, ALU.bypass)
                        acts.append(a_relu)
                    if h1 and t0 + 2 == h1:
                        def a_store1():
                            nc.sync.dma_start(
                                out=out_t[:, gt0 * D_OUT:(gt0 + h1) * D_OUT],
                                in_=og_p[:, 0:h1, :].rearrange(
                                    "p t d -> p (t d)"))
                        acts.append(a_store1)

                def a_store2():
                    nc.sync.dma_start(
                        out=out_t[:, (gt0 + h1) * D_OUT:(gt0 + gs) * D_OUT],
                        in_=og_p[:, h1:gs, :].rearrange("p t d -> p (t d)"))
                acts.append(a_store2)
                return acts

            prev_group = None
            for gi in range(len(schedule)):
                gt0, gs = schedule[gi]
                n_pairs = gs // 2
                xg = xloadp.tile([128, GSIZE * 128], F16, tag="xg")
                nc.sync.dma_start(out=xg[:, 0:gs * 128],
                                  in_=xin_c[:, gt0 * 128:(gt0 + gs) * 128])
                pg = ploadp.tile([128, GSIZE, D_OUT], U8, tag="pg")
                nc.sync.dma_start(
                    out=pg[:, 0:gs, :].rearrange("p t d -> p (t d)"),
                    in_=prin_t[:, gt0 * D_OUT:(gt0 + gs) * D_OUT])
                if gi == 0:
                    nc.sync.dma_start(out=jc_sb[:], in_=jc[:, :])
                    nc.sync.dma_start(out=sm_sb[:], in_=sm[:, :])
                og = outsp.tile([128, GSIZE, D_OUT], U8)

                stats = statsp.tile([128, GSIZE * SLOTS], F32)
                cums = statsp.tile([128, GSIZE * SLOTS], F32, tag="cums")
                pairs = statsp.tile([128, GSIZE * 81], F32, tag="pairs")
                ntaus = statsp.tile([128, GSIZE * 81], F32, tag="ntaus")
                ntau = smallp.tile([128, GSIZE], F32, tag="ntau")
                if gi < 3:
                    # zero slots 0/9 of every tile segment once per ring
                    # buffer (stats pool has 3 bufs); never written again
                    nc.gpsimd.memset(stats[:], 0.0)

                # interleave schedule for prev-group actions: after pair 0
                # emit scan+TT, after pair 1 STT+reduce, then pass2 chunks
                prev_math = math_actions(prev_group) if (
                    PIPE and prev_group is not None) else []
                prev_p2 = pass2_actions(prev_group) if (
                    PIPE and prev_group is not None) else []

                stt_at = min(3, n_pairs - 1)

                def run_chunk(pr):
                    if not PIPE or (not prev_math and not prev_p2):
                        return
                    if pr == 0:
                        # scan (DVE) + pairs-TT (GPSIMD) early so the TT
                        # result is ready when the STT needs it
                        for a in prev_math[0:2]:
                            a()
                        del prev_math[0:2]
                    elif pr == stt_at:
                        for a in prev_math:
                            a()
                        prev_math.clear()
                    elif pr > stt_at:
                        k = max(1, (len(prev_p2) + n_pairs - pr - 1)
                                // (n_pairs - pr))
                        for a in prev_p2[0:k]:
                            a()
                        del prev_p2[0:k]

                ztiles = []
                for pr in range(n_pairs):
                    t0 = 2 * pr
                    z_ps = psumz.tile([128, 2 * D_OUT], F32)
                    nc.tensor.matmul(z_ps[:], ones_sb[:], bp2_sb[:],
                                     start=True, stop=False)
                    nc.tensor.matmul(z_ps[:, 0:D_OUT],
                                     xg[:, t0 * 128:(t0 + 1) * 128],
                                     wp_sb[:], start=False, stop=True,
                                     skip_group_check=True)
                    nc.tensor.matmul(z_ps[:, D_OUT:2 * D_OUT],
                                     xg[:, (t0 + 1) * 128:(t0 + 2) * 128],
                                     wp_sb[:], start=False, stop=True,
                                     skip_group_check=True)

                    z_sb = zp.tile([128, 2 * D_OUT], F32)
                    pgp = pg[:, t0:t0 + 2, :].rearrange("p t d -> p (t d)")
                    mode = MULT_PAT[pr % len(MULT_PAT)]
                    if mode == "d":
                        nc.vector.tensor_tensor(z_sb[:], z_ps[:], pgp,
                                                ALU.mult)
                    else:
                        zc = zcp.tile([128, 2 * D_OUT], F32, tag="zc")
                        if mode == "m":
                            nc.sync.dma_start(out=zc[:], in_=z_ps[:])
                        else:
                            nc.scalar.copy(zc[:], z_ps[:])
                        nc.gpsimd.tensor_tensor(z_sb[:], zc[:], pgp, ALU.mult)

                    for h in range(2):
                        t = t0 + h
                        zt = z_sb[:, h * D_OUT:(h + 1) * D_OUT]
                        s0 = t * SLOTS
                        nc.vector.max(stats[:, s0 + 1:s0 + 9], zt[:, 0:128])
                        nc.vector.max(stats[:, s0 + 10:s0 + 18],
                                      zt[:, 128:256])
                    ztiles.append((t0, z_sb))
                    run_chunk(pr)

                for a in prev_math:
                    a()
                for a in prev_p2:
                    a()

                cur_group = ((gt0, gs), ztiles, og, ntau,
                             stats, cums, pairs, ntaus)
                if PIPE:
                    prev_group = cur_group
                else:
                    for a in math_actions(cur_group):
                        a()
                    for a in pass2_actions(cur_group):
                        a()

            if PIPE and prev_group is not None:
                for a in math_actions(prev_group):
                    a()
                for a in pass2_actions(prev_group):
                    a()

    _split_oversized_waits(nc)
    return nc


def _host_constants(W, gamma, beta, moving_mean, moving_var):
    inv = (gamma / np.sqrt(moving_var + 1e-3)).astype(np.float32)
    wp = (W * inv[None, :]).astype(F16NP)
    bp = (beta - moving_mean * inv).astype(F16NP).reshape(1, D_OUT)
    ones = np.ones((1, D_IN), dtype=F16NP)
    # jc[i, j] = -1/(i+j); (0,0) slot excluded by the reduce
    ij = np.add.outer(np.arange(9), np.arange(9)).astype(np.float32)
    ij[0, 0] = 1.0
    jrow = (-1.0 / ij).reshape(81).astype(np.float32)
    jrow[0] = 0.0
    jrow = np.tile(jrow, GSIZE)
    srow = np.tile(
        np.concatenate([[0.0], np.ones(8), [0.0], np.ones(8)]),
        GSIZE).astype(np.float32)
    jct = np.ascontiguousarray(np.broadcast_to(jrow, (128, len(jrow))),
                               dtype=np.float32)
    smt = np.ascontiguousarray(np.broadcast_to(srow, (128, len(srow))),
                               dtype=np.float32)
    return wp, bp, ones, jct, smt


_NC_CACHE = {}


def make_core_feeds(inputs, priors, W, gamma, beta, moving_mean, moving_var,
                    bc=BC, n_cores=N_CORES):
    inputs_t = np.ascontiguousarray(
        np.asarray(inputs, dtype=np.float32).T).astype(F16NP)  # [D_IN, B]
    pq = np.round(np.asarray(priors, dtype=np.float32) * 255.0).astype(np.uint8)
    n_tiles = bc // 128
    wp, bp, ones, jct, smt = _host_constants(
        np.asarray(W, dtype=np.float32), np.asarray(gamma, dtype=np.float32),
        np.asarray(beta, dtype=np.float32),
        np.asarray(moving_mean, dtype=np.float32),
        np.asarray(moving_var, dtype=np.float32))
    in_maps = []
    for c in range(n_cores):
        lo, hi = c * bc, (c + 1) * bc
        pr = np.ascontiguousarray(
            pq[lo:hi].reshape(n_tiles, 128, D_OUT).transpose(1, 0, 2)
        ).reshape(128, n_tiles * D_OUT)
        in_maps.append({
            "xin": np.ascontiguousarray(inputs_t[:, lo:hi]),
            "prin": pr,
            "wp": wp, "bp": bp, "ones": ones, "jc": jct, "sm": smt,
        })
    return in_maps


def kernel(inputs, priors, W, gamma, beta, moving_mean, moving_var):
    from concourse.bass_utils import run_bass_kernel_spmd

    in_maps = make_core_feeds(inputs, priors, W, gamma, beta,
                              moving_mean, moving_var)
    if BC not in _NC_CACHE:
        _NC_CACHE[BC] = build_nc(BC)
    nc = _NC_CACHE[BC]
    res = run_bass_kernel_spmd(nc, in_maps, list(range(N_CORES)))
    n_tiles = BC // 128
    parts = []
    inv255 = np.float32(1.0 / 255.0)
    for c in range(N_CORES):
        o = res.results[c]["out"].reshape(128, n_tiles, D_OUT)
        parts.append(
            o.transpose(1, 0, 2).reshape(BC, D_OUT).astype(np.float32) * inv255)
    return np.concatenate(parts, axis=0)
